# revision 26
# baseline (speedup 1.0000x reference)
"""Trainium2 Bass kernel for nn_CorrBlock: softmax(fmap1 @ fmap2.T / sqrt(D), axis=-1).

Sharding: fmap1 rows split across 8 cores (1024 rows each), fmap2 replicated.
Each core computes its [1024, 8192] slab of the output independently.

Device kernel (per core):
  - Inputs are pre-transposed on the host to [128, D/128, rows] so the
    contraction dim lands on SBUF partitions with no on-device transpose.
  - PE: matmuls accumulate the D=256 contraction in 2 chunks of 128 into PSUM.
  - ACT: Exp with fused 1/sqrt(D) scale reads PSUM, writes fp16 SBUF, and
    emits per-row partial sums via accum_out in the same pass.  ACT is the
    critical engine: 8.4M exps at 1 elem/lane/cycle @1.2GHz ~= 55us + ~260ns
    per-instruction overhead; everything else hides under its stream.
  - DVE: reciprocal of the row sum, then per-row scalar multiply in fp16.
  - Output DMA: one 2MB [128, 8192] fp16 transfer per row block (big DMAs
    sustain ~370+ GB/s vs ~260 for 512KB chunks); host upcasts to fp32.

Schedule:
  - ~2.8us of dummy matmuls on a memset scratch tile open the PE HAM clock
    gate (1.2 -> 2.4 GHz) while the input DMA streams in.
  - Phase A covers the input stream: blocks 0-2 process q0 in pieces sized
    to the staged f2 sub-transfers (256/768/1024 cols) so the ACT stream
    starts as soon as the first 256 f2 columns land; block 3's q0 pads the
    coverage until the q1 chunk arrives.
  - Phase B finishes each block row and streams its normalized 2MB out,
    spreading the output DMA across the whole run.
  - LAST BLOCK, to kill the end-of-run normalize+drain tail: its logits
    get a first PE pass whose PSUM chunks feed a DVE Schraudolph
    (bit-trick) exp + accumulate, giving the row sums to ~0.1% after a
    fixed calibration factor.  ACT then computes ln(sum) (same table set
    as exp), and the block's second PE pass feeds ACTIVATEs with
    bias = -ln(sum): the output leaves the ACT engine already normalized
    and each chunk DMAs immediately, so only one 512KB transfer remains
    after the final ACTIVATE instead of a 2MB mul+drain chain.
"""

import os
import sys

import numpy as np

if "/opt/trn_rl_repo" not in sys.path:
    sys.path.insert(0, "/opt/trn_rl_repo")

import concourse.bacc as bacc
import concourse.bass as bass
import concourse.mybir as mybir
import concourse.tile as tile
from concourse.bass_utils import run_bass_kernel_spmd

N, M, D = 8192, 8192, 256
N_CORES = 8
NB = N // N_CORES  # rows per core
DC = D // 128  # contraction chunks
QC = 2048  # columns handled per PSUM tile (4 banks)

MM_DT = os.environ.get("CORR_MM_DT", "float16")
N_WARM = int(os.environ.get("CORR_N_WARM", "20"))
N_SPLIT = int(os.environ.get("CORR_NSPLIT", "3"))  # blocks with split q0
LN_LAST = os.environ.get("CORR_LNLAST", "0") == "1"

# Schraudolph fast-exp constants for the last block's row sums, including
# the 1/sqrt(D) logit scale.  exp(x) ~= bitcast_f32(int32(x*SCH_A + SCH_B));
# the systematic bias of the row sum on N(0,1)-scaled logits calibrates to
# SCH_CORR (validated: residual <= 0.1% per row).
SCH_C = 321312.0
SCH_A = float(2**23 / np.log(2) / 16.0)
SCH_B = float(127 * 2**23 - SCH_C)
SCH_LNCORR = 0.0133414  # +ln(sum_correction); added to -ln(S~)

# Populated by kernel() on every run (exec_time_ns only when tracing).
last_run_info: dict = {}


def _chunks(m):
    """Uniform 2048-wide column chunks (4 PSUM banks each)."""
    if m % 2048:
        return [m]
    return [2048] * (m // 2048)


def build_nc(nb=NB, m=M, dc=DC, qc=QC, mm_dt=None):
    """Build the per-core Bass program. Shapes in elements."""
    f32 = mybir.dt.float32
    f16 = mybir.dt.float16
    i32 = mybir.dt.int32
    mm_dtype = getattr(mybir.dt, mm_dt or MM_DT)
    n_blocks = nb // 128
    chunks = _chunks(m)
    n_q = len(chunks)
    coff = [sum(chunks[:i]) for i in range(n_q + 1)]  # column offsets
    scale = 1.0 / (D**0.5)
    b_last = n_blocks - 1

    nc = bacc.Bacc("TRN2", target_bir_lowering=False, debug=False)

    f1t = nc.dram_tensor("f1t", [128, dc, nb], mm_dtype, kind="ExternalInput")
    f2t = nc.dram_tensor("f2t", [128, dc, m], mm_dtype, kind="ExternalInput")
    out = nc.dram_tensor("out", [nb, m], f16, kind="ExternalOutput")

    with tile.TileContext(nc) as tc:
        with (
            tc.tile_pool(name="weights", bufs=1) as wpool,
            tc.tile_pool(name="exps", bufs=n_blocks) as epool,
            tc.tile_pool(name="stats", bufs=n_blocks) as spool,
            tc.tile_pool(name="psum", bufs=2, space="PSUM") as ppool,
        ):
            # PE warm-up: the HAM clock gate only opens to 2.4 GHz after
            # ~3.4us of sustained PE activity.  Dummy matmuls on a memset
            # scratch tile (no DMA dependency) ramp the clock while the
            # input DMA streams in.
            ps_warm = ppool.tile([128, chunks[0]], f32, tag="ps")
            if N_WARM:
                scratch = wpool.tile([128, 128], mm_dtype, tag="scratch")
                nc.gpsimd.memset(scratch[:], 0)
                for w in range(N_WARM):
                    nc.tensor.matmul(
                        ps_warm[:, 0:128],
                        scratch[:],
                        scratch[:],
                        start=(w == 0),
                        stop=False,
                    )

            # Stage the input DMAs on the Sync HWDGE queue in consumption
            # order, interleaving f1 block pieces with f2 q0 sub-transfers
            # so each phase-A activation piece unblocks as early as
            # possible while the bulk f2 chunks stream behind.
            q0_splits = [0, 256, 1024, chunks[0]]  # f2 q0 sub-transfers
            f1s = wpool.tile([128, dc, nb], mm_dtype, tag="f1s")
            f2q0 = wpool.tile(
                [128, dc, chunks[0]], mm_dtype, tag="f2q0", name="f2q0"
            )
            # f1 rides the scalar engine's HWDGE ring (issued before any
            # ACTIVATE needs the queue), f2 rides the sync ring: the two
            # physical HWDGE rings stream inputs in parallel.
            nc.scalar.dma_start(f1s[:, :, 0:128], f1t[:, :, 0:128])
            nc.sync.dma_start(
                f2q0[:, :, q0_splits[0] : q0_splits[1]],
                f2t[:, :, q0_splits[0] : q0_splits[1]],
            )
            nc.scalar.dma_start(
                f1s[:, :, 128 : N_SPLIT * 128], f1t[:, :, 128 : N_SPLIT * 128]
            )
            nc.sync.dma_start(
                f2q0[:, :, q0_splits[1] : q0_splits[2]],
                f2t[:, :, q0_splits[1] : q0_splits[2]],
            )
            nc.sync.dma_start(
                f2q0[:, :, q0_splits[2] : q0_splits[3]],
                f2t[:, :, q0_splits[2] : q0_splits[3]],
            )
            nc.scalar.dma_start(
                f1s[:, :, N_SPLIT * 128 : nb], f1t[:, :, N_SPLIT * 128 : nb]
            )
            f2s = []
            for q in range(1, n_q):
                f2q = wpool.tile(
                    [128, dc, chunks[q]], mm_dtype, tag=f"f2q_{q}", name=f"f2q_{q}"
                )
                nc.sync.dma_start(f2q[:], f2t[:, :, coff[q] : coff[q + 1]])
                f2s.append(f2q)

            def rhs_slice(c0, c1, d):
                """RHS AP for matmul column group [c0, c1)."""
                if c1 <= chunks[0]:
                    return f2q0[:, d, c0:c1]
                q = c0 // qc
                r0, r1 = c0 % qc, c0 % qc + (c1 - c0)
                return f2s[q - 1][:, d, r0:r1]

            exps_t, sums_t, rsum_t, recip_t = [], [], [], []
            max_sums = n_q - 1 + len(q0_splits)
            for b in range(n_blocks):
                exps_t.append(epool.tile([128, m], f16, tag="exps", name=f"exps_{b}"))
                sums_t.append(
                    spool.tile([128, max_sums], f32, tag="sums", name=f"sums_{b}")
                )
                rsum_t.append(spool.tile([128, 1], f32, tag="rsum", name=f"rsum_{b}"))
                recip_t.append(
                    spool.tile([128, 1], f32, tag="recip", name=f"recip_{b}")
                )
            n_sums = [0] * n_blocks  # accumulator columns used per block

            def mm_chunk(b, c0, c1):
                """Matmul columns [c0,c1) of block b into a fresh ps tile.

                Column groups of <=512 (one PSUM bank each); group edges
                stay 512-aligned within the ps tile so each matmul output
                lands in a single bank."""
                w = c1 - c0
                ps = ppool.tile([128, w], f32, tag="ps", name=f"ps_{b}_{c0}")
                groups = []
                g0 = 0
                while g0 < w:
                    groups.append((g0, min(g0 + 512, w)))
                    g0 += 512
                for d in range(dc):
                    lhsT = f1s[:, d, b * 128 : (b + 1) * 128]
                    for j0, j1 in groups:
                        nc.tensor.matmul(
                            ps[:, j0:j1],
                            lhsT,
                            rhs_slice(c0 + j0, c0 + j1, d),
                            start=(d == 0),
                            stop=(d == dc - 1),
                        )
                return ps

            def do_cols(b, c0, c1):
                """Matmul columns [c0,c1) of block b + one Exp activation."""
                ps = mm_chunk(b, c0, c1)
                k = n_sums[b]
                n_sums[b] += 1
                nc.scalar.activation(
                    exps_t[b][:, c0:c1],
                    ps[:, :],
                    mybir.ActivationFunctionType.Exp,
                    scale=scale,
                    accum_out=sums_t[b][:, k : k + 1],
                )

            def normalize_and_store(b, n_dma=1):
                """Row sums -> reciprocal -> scale -> output DMA.  Mid-run
                blocks use one 2MB transfer (big DMAs sustain ~390 GB/s vs
                ~260 for 512KB chunks); the final blocks use finer pieces
                so their muls pipeline with the tail drain."""
                exps = exps_t[b]
                rsum, recip = rsum_t[b], recip_t[b]
                nc.vector.reduce_sum(
                    rsum[:], sums_t[b][:, 0 : n_sums[b]], axis=mybir.AxisListType.X
                )
                nc.vector.reciprocal(recip[:], rsum[:])
                per = n_q // n_dma  # mul chunks per DMA piece
                for q in range(n_q):
                    sl = slice(coff[q], coff[q + 1])
                    nc.vector.tensor_scalar_mul(exps[:, sl], exps[:, sl], recip[:])
                    if (q + 1) % per == 0:
                        d = slice(coff[q + 1 - per], coff[q + 1])
                        nc.sync.dma_start(
                            out[b * 128 : (b + 1) * 128, d], exps[:, d]
                        )

            # --- last-block (Ln-bias) machinery -------------------------
            if LN_LAST:
                sch_i32 = wpool.tile([128, chunks[0]], i32, tag="sch_i32")
                lnS = spool.tile([128, 1], f32, tag="lnS")
                neg_lnS = spool.tile([128, 1], f32, tag="neg_lnS")

            def last_pass1_chunk(q):
                """PE chunk -> DVE Schraudolph exp + accumulate row sum."""
                b = b_last
                c0, c1 = coff[q], coff[q + 1]
                ps = mm_chunk(b, c0, c1)
                w = c1 - c0
                # int32(raw_dot * A + B); bit pattern ~= fp32 exp(dot/16)
                nc.vector.tensor_scalar(
                    sch_i32[:, 0:w],
                    ps[:, :],
                    SCH_A,
                    SCH_B,
                    mybir.AluOpType.mult,
                    mybir.AluOpType.add,
                )
                # dummy scale pass over the bitcast fp32 terms to harvest
                # the row sum via the DVE accumulator
                k = n_sums[b]
                n_sums[b] += 1
                nc.vector.tensor_scalar(
                    exps_t[b][:, c0:c1],
                    sch_i32[:, 0:w].bitcast(f32),
                    1.0,
                    0.0,
                    mybir.AluOpType.mult,
                    mybir.AluOpType.add,
                    accum_out=sums_t[b][:, k : k + 1],
                )

            def last_prepare_bias():
                b = b_last
                nc.vector.reduce_sum(
                    rsum_t[b][:], sums_t[b][:, 0 : n_sums[b]],
                    axis=mybir.AxisListType.X,
                )
                nc.scalar.activation(
                    lnS[:], rsum_t[b][:], mybir.ActivationFunctionType.Ln
                )
                # -ln(S~) + ln(corr)
                nc.vector.tensor_scalar(
                    neg_lnS[:],
                    lnS[:],
                    -1.0,
                    SCH_LNCORR,
                    mybir.AluOpType.mult,
                    mybir.AluOpType.add,
                )

            def last_pass2_chunk(q):
                """PE chunk again -> ACT exp with bias=-ln(S): output is
                normalized as it leaves ACT; DMA the chunk immediately."""
                b = b_last
                c0, c1 = coff[q], coff[q + 1]
                ps = mm_chunk(b, c0, c1)
                nc.scalar.activation(
                    exps_t[b][:, c0:c1],
                    ps[:, :],
                    mybir.ActivationFunctionType.Exp,
                    scale=scale,
                    bias=neg_lnS[:],
                )
                nc.sync.dma_start(
                    out[b * 128 : (b + 1) * 128, c0:c1], exps_t[b][:, c0:c1]
                )

            # --- schedule ----------------------------------------------
            # Phase A: cover the input DMA stream.
            for i in range(len(q0_splits) - 1):
                for b in range(N_SPLIT):
                    do_cols(b, q0_splits[i], q0_splits[i + 1])
            for b in range(N_SPLIT, min(N_SPLIT + 1, n_blocks)):
                do_cols(b, 0, chunks[0])

            # Phase B: finish each block row, stream its output.  The last
            # block's pass-1 chunks are spread one at a time into the
            # middle of blocks 3-4 (away from the normalize bursts, so
            # their DVE work never holds a PSUM ring slot that the next
            # ACT chunk's matmuls are waiting for), and the bias prep runs
            # during block 5, long before pass 2 needs it.
            n_main = n_blocks - 1 if LN_LAST else n_blocks
            inserts = {}
            if LN_LAST:
                inserts = {
                    (3, 1): lambda: last_pass1_chunk(0),
                    (3, 2): lambda: last_pass1_chunk(1),
                    (4, 1): lambda: last_pass1_chunk(2),
                    (4, 2): lambda: last_pass1_chunk(3),
                    (5, 1): last_prepare_bias,
                }
            for b in range(n_main):
                q_start = 1 if b <= N_SPLIT else 0
                for q in range(q_start, n_q):
                    do_cols(b, coff[q], coff[q + 1])
                    ins = inserts.pop((b, q), None)
                    if ins is not None:
                        ins()
                if b >= n_main - 2:
                    n_dma = 4 if b == n_main - 1 else 2
                else:
                    n_dma = 1
                normalize_and_store(b, n_dma=n_dma)
            if LN_LAST:
                for q in range(n_q):
                    last_pass2_chunk(q)

    nc.compile()
    return nc


_nc_cache: dict = {}


def _get_nc():
    key = (MM_DT, N_WARM, N_SPLIT, LN_LAST)
    if key not in _nc_cache:
        _nc_cache[key] = build_nc()
    return _nc_cache[key]


def kernel(fmap1: np.ndarray, fmap2: np.ndarray) -> np.ndarray:
    f1 = np.asarray(fmap1, dtype=np.float32)
    f2 = np.asarray(fmap2, dtype=np.float32)
    np_mm = mybir.dt.np(getattr(mybir.dt, MM_DT))
    # [rows, D] -> [128, D/128, rows]: f1t[dp, dcc, n] = f1[n, dcc*128 + dp]
    f1t = np.ascontiguousarray(
        f1.T.reshape(DC, 128, N).transpose(1, 0, 2).astype(np_mm)
    )
    f2t = np.ascontiguousarray(
        f2.T.reshape(DC, 128, M).transpose(1, 0, 2).astype(np_mm)
    )

    nc = _get_nc()
    in_maps = [
        {"f1t": np.ascontiguousarray(f1t[:, :, i * NB : (i + 1) * NB]), "f2t": f2t}
        for i in range(N_CORES)
    ]
    trace = bool(os.environ.get("BASS_TRACE"))
    res = run_bass_kernel_spmd(nc, in_maps, list(range(N_CORES)), trace=trace)
    last_run_info.clear()
    last_run_info.update(
        exec_time_ns=res.exec_time_ns,
        mean_exec_time_ns=res.mean_exec_time_ns,
        profile_json=res.profile_json,
        trace_path=(res.instructions_and_trace or (None, None))[1],
    )
    return np.concatenate(
        [res.results[i]["out"] for i in range(N_CORES)], axis=0
    ).astype(np.float32)


# revision 27
# speedup vs baseline: 1.0171x; 1.0171x over previous
"""Trainium2 Bass kernel for nn_CorrBlock: softmax(fmap1 @ fmap2.T / sqrt(D), axis=-1).

Sharding: fmap1 rows split across 8 cores (1024 rows each), fmap2 replicated.
Each core computes its [1024, 8192] slab of the output independently.

Device kernel (per core):
  - Inputs are pre-transposed on the host to [128, D/128, rows] so the
    contraction dim lands on SBUF partitions with no on-device transpose.
  - PE: matmuls accumulate the D=256 contraction in 2 chunks of 128 into PSUM.
  - ACT: Exp with fused 1/sqrt(D) scale reads PSUM, writes fp16 SBUF, and
    emits per-row partial sums via accum_out in the same pass.  ACT is the
    critical engine: 8.4M exps at 1 elem/lane/cycle @1.2GHz ~= 55us + ~260ns
    per-instruction overhead; everything else hides under its stream.
  - DVE: reciprocal of the row sum, then per-row scalar multiply in fp16.
  - Output DMA: one 2MB [128, 8192] fp16 transfer per row block (big DMAs
    sustain ~370+ GB/s vs ~260 for 512KB chunks); host upcasts to fp32.

Schedule:
  - ~2.8us of dummy matmuls on a memset scratch tile open the PE HAM clock
    gate (1.2 -> 2.4 GHz) while the input DMA streams in.
  - Phase A covers the input stream: blocks 0-2 process q0 in pieces sized
    to the staged f2 sub-transfers (256/768/1024 cols) so the ACT stream
    starts as soon as the first 256 f2 columns land; block 3's q0 pads the
    coverage until the q1 chunk arrives.
  - Phase B finishes each block row and streams its normalized 2MB out,
    spreading the output DMA across the whole run.
  - LAST BLOCK, to kill the end-of-run normalize+drain tail: its logits
    get a first PE pass whose PSUM chunks feed a DVE Schraudolph
    (bit-trick) exp + accumulate, giving the row sums to ~0.1% after a
    fixed calibration factor.  ACT then computes ln(sum) (same table set
    as exp), and the block's second PE pass feeds ACTIVATEs with
    bias = -ln(sum): the output leaves the ACT engine already normalized
    and each chunk DMAs immediately, so only one 512KB transfer remains
    after the final ACTIVATE instead of a 2MB mul+drain chain.
"""

import os
import sys

import numpy as np

if "/opt/trn_rl_repo" not in sys.path:
    sys.path.insert(0, "/opt/trn_rl_repo")

import concourse.bacc as bacc
import concourse.bass as bass
import concourse.mybir as mybir
import concourse.tile as tile
from concourse.bass_utils import run_bass_kernel_spmd

N, M, D = 8192, 8192, 256
N_CORES = 8
NB = N // N_CORES  # rows per core
DC = D // 128  # contraction chunks
QC = 2048  # columns handled per PSUM tile (4 banks)

MM_DT = os.environ.get("CORR_MM_DT", "float16")
N_WARM = int(os.environ.get("CORR_N_WARM", "20"))
N_SPLIT = int(os.environ.get("CORR_NSPLIT", "3"))  # blocks with split q0
LN_LAST = os.environ.get("CORR_LNLAST", "0") == "1"

# Schraudolph fast-exp constants for the last block's row sums, including
# the 1/sqrt(D) logit scale.  exp(x) ~= bitcast_f32(int32(x*SCH_A + SCH_B));
# the systematic bias of the row sum on N(0,1)-scaled logits calibrates to
# SCH_CORR (validated: residual <= 0.1% per row).
SCH_C = 321312.0
SCH_A = float(2**23 / np.log(2) / 16.0)
SCH_B = float(127 * 2**23 - SCH_C)
SCH_LNCORR = 0.0133414  # +ln(sum_correction); added to -ln(S~)

# Populated by kernel() on every run (exec_time_ns only when tracing).
last_run_info: dict = {}


def _chunks(m):
    """Uniform 2048-wide column chunks (4 PSUM banks each)."""
    if m % 2048:
        return [m]
    return [2048] * (m // 2048)


def build_nc(nb=NB, m=M, dc=DC, qc=QC, mm_dt=None):
    """Build the per-core Bass program. Shapes in elements."""
    f32 = mybir.dt.float32
    f16 = mybir.dt.float16
    i32 = mybir.dt.int32
    mm_dtype = getattr(mybir.dt, mm_dt or MM_DT)
    n_blocks = nb // 128
    chunks = _chunks(m)
    n_q = len(chunks)
    coff = [sum(chunks[:i]) for i in range(n_q + 1)]  # column offsets
    scale = 1.0 / (D**0.5)
    b_last = n_blocks - 1

    nc = bacc.Bacc("TRN2", target_bir_lowering=False, debug=False)

    f1t = nc.dram_tensor("f1t", [128, dc, nb], mm_dtype, kind="ExternalInput")
    f2t = nc.dram_tensor("f2t", [128, dc, m], mm_dtype, kind="ExternalInput")
    out = nc.dram_tensor("out", [nb, m], f16, kind="ExternalOutput")

    with tile.TileContext(nc) as tc:
        with (
            tc.tile_pool(name="weights", bufs=1) as wpool,
            tc.tile_pool(name="exps", bufs=n_blocks) as epool,
            tc.tile_pool(name="stats", bufs=n_blocks) as spool,
            tc.tile_pool(name="psum", bufs=2, space="PSUM") as ppool,
        ):
            # PE warm-up: the HAM clock gate only opens to 2.4 GHz after
            # ~3.4us of sustained PE activity.  Dummy matmuls on a memset
            # scratch tile (no DMA dependency) ramp the clock while the
            # input DMA streams in.
            ps_warm = ppool.tile([128, chunks[0]], f32, tag="ps")
            if N_WARM:
                scratch = wpool.tile([128, 128], mm_dtype, tag="scratch")
                nc.gpsimd.memset(scratch[:], 0)
                for w in range(N_WARM):
                    nc.tensor.matmul(
                        ps_warm[:, 0:128],
                        scratch[:],
                        scratch[:],
                        start=(w == 0),
                        stop=False,
                    )

            # Stage the input DMAs on the Sync HWDGE queue in consumption
            # order, interleaving f1 block pieces with f2 q0 sub-transfers
            # so each phase-A activation piece unblocks as early as
            # possible while the bulk f2 chunks stream behind.
            q0_splits = [0, 256, 1024, chunks[0]]  # f2 q0 sub-transfers
            f1s = wpool.tile([128, dc, nb], mm_dtype, tag="f1s")
            f2q0 = wpool.tile(
                [128, dc, chunks[0]], mm_dtype, tag="f2q0", name="f2q0"
            )
            nc.sync.dma_start(f1s[:, :, 0:128], f1t[:, :, 0:128])
            nc.sync.dma_start(
                f2q0[:, :, q0_splits[0] : q0_splits[1]],
                f2t[:, :, q0_splits[0] : q0_splits[1]],
            )
            nc.sync.dma_start(
                f1s[:, :, 128 : N_SPLIT * 128], f1t[:, :, 128 : N_SPLIT * 128]
            )
            nc.sync.dma_start(
                f2q0[:, :, q0_splits[1] : q0_splits[2]],
                f2t[:, :, q0_splits[1] : q0_splits[2]],
            )
            nc.sync.dma_start(
                f2q0[:, :, q0_splits[2] : q0_splits[3]],
                f2t[:, :, q0_splits[2] : q0_splits[3]],
            )
            nc.sync.dma_start(
                f1s[:, :, N_SPLIT * 128 : nb], f1t[:, :, N_SPLIT * 128 : nb]
            )
            f2s = []
            for q in range(1, n_q):
                f2q = wpool.tile(
                    [128, dc, chunks[q]], mm_dtype, tag=f"f2q_{q}", name=f"f2q_{q}"
                )
                nc.sync.dma_start(f2q[:], f2t[:, :, coff[q] : coff[q + 1]])
                f2s.append(f2q)

            def rhs_slice(c0, c1, d):
                """RHS AP for matmul column group [c0, c1)."""
                if c1 <= chunks[0]:
                    return f2q0[:, d, c0:c1]
                q = c0 // qc
                r0, r1 = c0 % qc, c0 % qc + (c1 - c0)
                return f2s[q - 1][:, d, r0:r1]

            exps_t, sums_t, rsum_t, recip_t = [], [], [], []
            max_sums = n_q - 1 + len(q0_splits)
            for b in range(n_blocks):
                exps_t.append(epool.tile([128, m], f16, tag="exps", name=f"exps_{b}"))
                sums_t.append(
                    spool.tile([128, max_sums], f32, tag="sums", name=f"sums_{b}")
                )
                rsum_t.append(spool.tile([128, 1], f32, tag="rsum", name=f"rsum_{b}"))
                recip_t.append(
                    spool.tile([128, 1], f32, tag="recip", name=f"recip_{b}")
                )
            n_sums = [0] * n_blocks  # accumulator columns used per block

            def mm_chunk(b, c0, c1):
                """Matmul columns [c0,c1) of block b into a fresh ps tile.

                Column groups of <=512 (one PSUM bank each); group edges
                stay 512-aligned within the ps tile so each matmul output
                lands in a single bank."""
                w = c1 - c0
                ps = ppool.tile([128, w], f32, tag="ps", name=f"ps_{b}_{c0}")
                groups = []
                g0 = 0
                while g0 < w:
                    groups.append((g0, min(g0 + 512, w)))
                    g0 += 512
                for d in range(dc):
                    lhsT = f1s[:, d, b * 128 : (b + 1) * 128]
                    for j0, j1 in groups:
                        nc.tensor.matmul(
                            ps[:, j0:j1],
                            lhsT,
                            rhs_slice(c0 + j0, c0 + j1, d),
                            start=(d == 0),
                            stop=(d == dc - 1),
                        )
                return ps

            def do_cols(b, c0, c1):
                """Matmul columns [c0,c1) of block b + one Exp activation."""
                ps = mm_chunk(b, c0, c1)
                k = n_sums[b]
                n_sums[b] += 1
                nc.scalar.activation(
                    exps_t[b][:, c0:c1],
                    ps[:, :],
                    mybir.ActivationFunctionType.Exp,
                    scale=scale,
                    accum_out=sums_t[b][:, k : k + 1],
                )

            def normalize_and_store(b, n_dma=1):
                """Row sums -> reciprocal -> scale -> output DMA.  Mid-run
                blocks use one 2MB transfer (big DMAs sustain ~390 GB/s vs
                ~260 for 512KB chunks); the final blocks use finer pieces
                so their muls pipeline with the tail drain."""
                exps = exps_t[b]
                rsum, recip = rsum_t[b], recip_t[b]
                nc.vector.reduce_sum(
                    rsum[:], sums_t[b][:, 0 : n_sums[b]], axis=mybir.AxisListType.X
                )
                nc.vector.reciprocal(recip[:], rsum[:])
                per = n_q // n_dma  # mul chunks per DMA piece
                for q in range(n_q):
                    sl = slice(coff[q], coff[q + 1])
                    nc.vector.tensor_scalar_mul(exps[:, sl], exps[:, sl], recip[:])
                    if (q + 1) % per == 0:
                        d = slice(coff[q + 1 - per], coff[q + 1])
                        nc.sync.dma_start(
                            out[b * 128 : (b + 1) * 128, d], exps[:, d]
                        )

            # --- last-block (Ln-bias) machinery -------------------------
            if LN_LAST:
                sch_i32 = wpool.tile([128, chunks[0]], i32, tag="sch_i32")
                lnS = spool.tile([128, 1], f32, tag="lnS")
                neg_lnS = spool.tile([128, 1], f32, tag="neg_lnS")

            def last_pass1_chunk(q):
                """PE chunk -> DVE Schraudolph exp + accumulate row sum."""
                b = b_last
                c0, c1 = coff[q], coff[q + 1]
                ps = mm_chunk(b, c0, c1)
                w = c1 - c0
                # int32(raw_dot * A + B); bit pattern ~= fp32 exp(dot/16)
                nc.vector.tensor_scalar(
                    sch_i32[:, 0:w],
                    ps[:, :],
                    SCH_A,
                    SCH_B,
                    mybir.AluOpType.mult,
                    mybir.AluOpType.add,
                )
                # dummy scale pass over the bitcast fp32 terms to harvest
                # the row sum via the DVE accumulator
                k = n_sums[b]
                n_sums[b] += 1
                nc.vector.tensor_scalar(
                    exps_t[b][:, c0:c1],
                    sch_i32[:, 0:w].bitcast(f32),
                    1.0,
                    0.0,
                    mybir.AluOpType.mult,
                    mybir.AluOpType.add,
                    accum_out=sums_t[b][:, k : k + 1],
                )

            def last_prepare_bias():
                b = b_last
                nc.vector.reduce_sum(
                    rsum_t[b][:], sums_t[b][:, 0 : n_sums[b]],
                    axis=mybir.AxisListType.X,
                )
                nc.scalar.activation(
                    lnS[:], rsum_t[b][:], mybir.ActivationFunctionType.Ln
                )
                # -ln(S~) + ln(corr)
                nc.vector.tensor_scalar(
                    neg_lnS[:],
                    lnS[:],
                    -1.0,
                    SCH_LNCORR,
                    mybir.AluOpType.mult,
                    mybir.AluOpType.add,
                )

            def last_pass2_chunk(q):
                """PE chunk again -> ACT exp with bias=-ln(S): output is
                normalized as it leaves ACT; DMA the chunk immediately."""
                b = b_last
                c0, c1 = coff[q], coff[q + 1]
                ps = mm_chunk(b, c0, c1)
                nc.scalar.activation(
                    exps_t[b][:, c0:c1],
                    ps[:, :],
                    mybir.ActivationFunctionType.Exp,
                    scale=scale,
                    bias=neg_lnS[:],
                )
                nc.sync.dma_start(
                    out[b * 128 : (b + 1) * 128, c0:c1], exps_t[b][:, c0:c1]
                )

            # --- schedule ----------------------------------------------
            # Phase A: cover the input DMA stream.
            for i in range(len(q0_splits) - 1):
                for b in range(N_SPLIT):
                    do_cols(b, q0_splits[i], q0_splits[i + 1])
            for b in range(N_SPLIT, min(N_SPLIT + 1, n_blocks)):
                do_cols(b, 0, chunks[0])

            # Phase B: finish each block row, stream its output.  The last
            # block's pass-1 chunks are spread one at a time into the
            # middle of blocks 3-4 (away from the normalize bursts, so
            # their DVE work never holds a PSUM ring slot that the next
            # ACT chunk's matmuls are waiting for), and the bias prep runs
            # during block 5, long before pass 2 needs it.
            n_main = n_blocks - 1 if LN_LAST else n_blocks
            inserts = {}
            if LN_LAST:
                inserts = {
                    (3, 1): lambda: last_pass1_chunk(0),
                    (3, 2): lambda: last_pass1_chunk(1),
                    (4, 1): lambda: last_pass1_chunk(2),
                    (4, 2): lambda: last_pass1_chunk(3),
                    (5, 1): last_prepare_bias,
                }
            for b in range(n_main):
                q_start = 1 if b <= N_SPLIT else 0
                for q in range(q_start, n_q):
                    do_cols(b, coff[q], coff[q + 1])
                    ins = inserts.pop((b, q), None)
                    if ins is not None:
                        ins()
                if b >= n_main - 2:
                    n_dma = 4 if b == n_main - 1 else 2
                else:
                    n_dma = 1
                normalize_and_store(b, n_dma=n_dma)
            if LN_LAST:
                for q in range(n_q):
                    last_pass2_chunk(q)

    nc.compile()
    return nc


_nc_cache: dict = {}


def _get_nc():
    key = (MM_DT, N_WARM, N_SPLIT, LN_LAST)
    if key not in _nc_cache:
        _nc_cache[key] = build_nc()
    return _nc_cache[key]


def kernel(fmap1: np.ndarray, fmap2: np.ndarray) -> np.ndarray:
    f1 = np.asarray(fmap1, dtype=np.float32)
    f2 = np.asarray(fmap2, dtype=np.float32)
    np_mm = mybir.dt.np(getattr(mybir.dt, MM_DT))
    # [rows, D] -> [128, D/128, rows]: f1t[dp, dcc, n] = f1[n, dcc*128 + dp]
    f1t = np.ascontiguousarray(
        f1.T.reshape(DC, 128, N).transpose(1, 0, 2).astype(np_mm)
    )
    f2t = np.ascontiguousarray(
        f2.T.reshape(DC, 128, M).transpose(1, 0, 2).astype(np_mm)
    )

    nc = _get_nc()
    in_maps = [
        {"f1t": np.ascontiguousarray(f1t[:, :, i * NB : (i + 1) * NB]), "f2t": f2t}
        for i in range(N_CORES)
    ]
    trace = bool(os.environ.get("BASS_TRACE"))
    res = run_bass_kernel_spmd(nc, in_maps, list(range(N_CORES)), trace=trace)
    last_run_info.clear()
    last_run_info.update(
        exec_time_ns=res.exec_time_ns,
        mean_exec_time_ns=res.mean_exec_time_ns,
        profile_json=res.profile_json,
        trace_path=(res.instructions_and_trace or (None, None))[1],
    )
    return np.concatenate(
        [res.results[i]["out"] for i in range(N_CORES)], axis=0
    ).astype(np.float32)


# revision 28
# speedup vs baseline: 1.0340x; 1.0166x over previous
"""Trainium2 Bass kernel for nn_CorrBlock: softmax(fmap1 @ fmap2.T / sqrt(D), axis=-1).

Sharding: fmap1 rows split across 8 cores (1024 rows each), fmap2 replicated.
Each core computes its [1024, 8192] slab of the output independently.

Device kernel (per core):
  - Inputs are pre-transposed on the host to [128, D/128, rows] so the
    contraction dim lands on SBUF partitions with no on-device transpose.
  - PE: matmuls accumulate the D=256 contraction in 2 chunks of 128 into PSUM.
  - ACT: Exp with fused 1/sqrt(D) scale reads PSUM, writes fp16 SBUF, and
    emits per-row partial sums via accum_out in the same pass.  ACT is the
    critical engine: 8.4M exps at 1 elem/lane/cycle @1.2GHz ~= 55us + ~260ns
    per-instruction overhead; everything else hides under its stream.
  - DVE: reciprocal of the row sum, then per-row scalar multiply in fp16.
  - Output DMA: one 2MB [128, 8192] fp16 transfer per row block (big DMAs
    sustain ~370+ GB/s vs ~260 for 512KB chunks); host upcasts to fp32.

Schedule:
  - ~2.8us of dummy matmuls on a memset scratch tile open the PE HAM clock
    gate (1.2 -> 2.4 GHz) while the input DMA streams in.
  - Phase A covers the input stream: blocks 0-2 process q0 in pieces sized
    to the staged f2 sub-transfers (256/768/1024 cols) so the ACT stream
    starts as soon as the first 256 f2 columns land; block 3's q0 pads the
    coverage until the q1 chunk arrives.
  - Phase B finishes each block row and streams its normalized 2MB out,
    spreading the output DMA across the whole run.
  - LAST BLOCK, to kill the end-of-run normalize+drain tail: its logits
    get a first PE pass whose PSUM chunks feed a DVE Schraudolph
    (bit-trick) exp + accumulate, giving the row sums to ~0.1% after a
    fixed calibration factor.  ACT then computes ln(sum) (same table set
    as exp), and the block's second PE pass feeds ACTIVATEs with
    bias = -ln(sum): the output leaves the ACT engine already normalized
    and each chunk DMAs immediately, so only one 512KB transfer remains
    after the final ACTIVATE instead of a 2MB mul+drain chain.
"""

import os
import sys

import numpy as np

if "/opt/trn_rl_repo" not in sys.path:
    sys.path.insert(0, "/opt/trn_rl_repo")

import concourse.bacc as bacc
import concourse.bass as bass
import concourse.mybir as mybir
import concourse.tile as tile
from concourse.bass_utils import run_bass_kernel_spmd

N, M, D = 8192, 8192, 256
N_CORES = 8
NB = N // N_CORES  # rows per core
DC = D // 128  # contraction chunks
QC = 2048  # columns handled per PSUM tile (4 banks)

MM_DT = os.environ.get("CORR_MM_DT", "float16")
N_WARM = int(os.environ.get("CORR_N_WARM", "20"))
N_SPLIT = int(os.environ.get("CORR_NSPLIT", "3"))  # blocks with split q0
LN_LAST = os.environ.get("CORR_LNLAST", "0") == "1"

# Schraudolph fast-exp constants for the last block's row sums, including
# the 1/sqrt(D) logit scale.  exp(x) ~= bitcast_f32(int32(x*SCH_A + SCH_B));
# the systematic bias of the row sum on N(0,1)-scaled logits calibrates to
# SCH_CORR (validated: residual <= 0.1% per row).
SCH_C = 321312.0
SCH_A = float(2**23 / np.log(2) / 16.0)
SCH_B = float(127 * 2**23 - SCH_C)
SCH_LNCORR = 0.0133414  # +ln(sum_correction); added to -ln(S~)

# Populated by kernel() on every run (exec_time_ns only when tracing).
last_run_info: dict = {}


def _chunks(m):
    """Uniform 2048-wide column chunks (4 PSUM banks each)."""
    if m % 2048:
        return [m]
    return [2048] * (m // 2048)


def build_nc(nb=NB, m=M, dc=DC, qc=QC, mm_dt=None):
    """Build the per-core Bass program. Shapes in elements."""
    f32 = mybir.dt.float32
    f16 = mybir.dt.float16
    i32 = mybir.dt.int32
    mm_dtype = getattr(mybir.dt, mm_dt or MM_DT)
    n_blocks = nb // 128
    chunks = _chunks(m)
    n_q = len(chunks)
    coff = [sum(chunks[:i]) for i in range(n_q + 1)]  # column offsets
    scale = 1.0 / (D**0.5)
    b_last = n_blocks - 1

    nc = bacc.Bacc("TRN2", target_bir_lowering=False, debug=False)

    f1t = nc.dram_tensor("f1t", [128, dc, nb], mm_dtype, kind="ExternalInput")
    f2t = nc.dram_tensor("f2t", [128, dc, m], mm_dtype, kind="ExternalInput")
    out = nc.dram_tensor("out", [nb, m], f16, kind="ExternalOutput")

    with tile.TileContext(nc) as tc:
        with (
            tc.tile_pool(name="weights", bufs=1) as wpool,
            tc.tile_pool(name="exps", bufs=n_blocks) as epool,
            tc.tile_pool(name="stats", bufs=n_blocks) as spool,
            tc.tile_pool(name="psum", bufs=2, space="PSUM") as ppool,
        ):
            # PE warm-up: the HAM clock gate only opens to 2.4 GHz after
            # ~3.4us of sustained PE activity.  Dummy matmuls on a memset
            # scratch tile (no DMA dependency) ramp the clock while the
            # input DMA streams in.
            ps_warm = ppool.tile([128, chunks[0]], f32, tag="ps")
            if N_WARM:
                scratch = wpool.tile([128, 128], mm_dtype, tag="scratch")
                nc.gpsimd.memset(scratch[:], 0)
                for w in range(N_WARM):
                    nc.tensor.matmul(
                        ps_warm[:, 0:128],
                        scratch[:],
                        scratch[:],
                        start=(w == 0),
                        stop=False,
                    )

            # Stage the input DMAs on the Sync HWDGE queue in consumption
            # order, interleaving f1 block pieces with f2 q0 sub-transfers
            # so each phase-A activation piece unblocks as early as
            # possible while the bulk f2 chunks stream behind.
            q0_splits = [0, 256, 768, chunks[0]]  # f2 q0 sub-transfers
            f1s = wpool.tile([128, dc, nb], mm_dtype, tag="f1s")
            f2q0 = wpool.tile(
                [128, dc, chunks[0]], mm_dtype, tag="f2q0", name="f2q0"
            )
            nc.sync.dma_start(f1s[:, :, 0:128], f1t[:, :, 0:128])
            nc.sync.dma_start(
                f2q0[:, :, q0_splits[0] : q0_splits[1]],
                f2t[:, :, q0_splits[0] : q0_splits[1]],
            )
            nc.sync.dma_start(
                f1s[:, :, 128 : N_SPLIT * 128], f1t[:, :, 128 : N_SPLIT * 128]
            )
            nc.sync.dma_start(
                f2q0[:, :, q0_splits[1] : q0_splits[2]],
                f2t[:, :, q0_splits[1] : q0_splits[2]],
            )
            nc.sync.dma_start(
                f2q0[:, :, q0_splits[2] : q0_splits[3]],
                f2t[:, :, q0_splits[2] : q0_splits[3]],
            )
            nc.sync.dma_start(
                f1s[:, :, N_SPLIT * 128 : nb], f1t[:, :, N_SPLIT * 128 : nb]
            )
            f2s = []
            for q in range(1, n_q):
                f2q = wpool.tile(
                    [128, dc, chunks[q]], mm_dtype, tag=f"f2q_{q}", name=f"f2q_{q}"
                )
                nc.sync.dma_start(f2q[:], f2t[:, :, coff[q] : coff[q + 1]])
                f2s.append(f2q)

            def rhs_slice(c0, c1, d):
                """RHS AP for matmul column group [c0, c1)."""
                if c1 <= chunks[0]:
                    return f2q0[:, d, c0:c1]
                q = c0 // qc
                r0, r1 = c0 % qc, c0 % qc + (c1 - c0)
                return f2s[q - 1][:, d, r0:r1]

            exps_t, sums_t, rsum_t, recip_t = [], [], [], []
            max_sums = n_q - 1 + len(q0_splits)
            for b in range(n_blocks):
                exps_t.append(epool.tile([128, m], f16, tag="exps", name=f"exps_{b}"))
                sums_t.append(
                    spool.tile([128, max_sums], f32, tag="sums", name=f"sums_{b}")
                )
                rsum_t.append(spool.tile([128, 1], f32, tag="rsum", name=f"rsum_{b}"))
                recip_t.append(
                    spool.tile([128, 1], f32, tag="recip", name=f"recip_{b}")
                )
            n_sums = [0] * n_blocks  # accumulator columns used per block

            def mm_chunk(b, c0, c1):
                """Matmul columns [c0,c1) of block b into a fresh ps tile.

                Column groups of <=512 (one PSUM bank each); group edges
                stay 512-aligned within the ps tile so each matmul output
                lands in a single bank."""
                w = c1 - c0
                ps = ppool.tile([128, w], f32, tag="ps", name=f"ps_{b}_{c0}")
                groups = []
                g0 = 0
                while g0 < w:
                    groups.append((g0, min(g0 + 512, w)))
                    g0 += 512
                for d in range(dc):
                    lhsT = f1s[:, d, b * 128 : (b + 1) * 128]
                    for j0, j1 in groups:
                        nc.tensor.matmul(
                            ps[:, j0:j1],
                            lhsT,
                            rhs_slice(c0 + j0, c0 + j1, d),
                            start=(d == 0),
                            stop=(d == dc - 1),
                        )
                return ps

            def do_cols(b, c0, c1):
                """Matmul columns [c0,c1) of block b + one Exp activation."""
                ps = mm_chunk(b, c0, c1)
                k = n_sums[b]
                n_sums[b] += 1
                nc.scalar.activation(
                    exps_t[b][:, c0:c1],
                    ps[:, :],
                    mybir.ActivationFunctionType.Exp,
                    scale=scale,
                    accum_out=sums_t[b][:, k : k + 1],
                )

            def normalize_and_store(b, n_dma=1):
                """Row sums -> reciprocal -> scale -> output DMA.  Mid-run
                blocks use one 2MB transfer (big DMAs sustain ~390 GB/s vs
                ~260 for 512KB chunks); the final blocks use finer pieces
                so their muls pipeline with the tail drain."""
                exps = exps_t[b]
                rsum, recip = rsum_t[b], recip_t[b]
                nc.vector.reduce_sum(
                    rsum[:], sums_t[b][:, 0 : n_sums[b]], axis=mybir.AxisListType.X
                )
                nc.vector.reciprocal(recip[:], rsum[:])
                per = n_q // n_dma  # mul chunks per DMA piece
                for q in range(n_q):
                    sl = slice(coff[q], coff[q + 1])
                    nc.vector.tensor_scalar_mul(exps[:, sl], exps[:, sl], recip[:])
                    if (q + 1) % per == 0:
                        d = slice(coff[q + 1 - per], coff[q + 1])
                        nc.sync.dma_start(
                            out[b * 128 : (b + 1) * 128, d], exps[:, d]
                        )

            # --- last-block (Ln-bias) machinery -------------------------
            if LN_LAST:
                sch_i32 = wpool.tile([128, chunks[0]], i32, tag="sch_i32")
                lnS = spool.tile([128, 1], f32, tag="lnS")
                neg_lnS = spool.tile([128, 1], f32, tag="neg_lnS")

            def last_pass1_chunk(q):
                """PE chunk -> DVE Schraudolph exp + accumulate row sum."""
                b = b_last
                c0, c1 = coff[q], coff[q + 1]
                ps = mm_chunk(b, c0, c1)
                w = c1 - c0
                # int32(raw_dot * A + B); bit pattern ~= fp32 exp(dot/16)
                nc.vector.tensor_scalar(
                    sch_i32[:, 0:w],
                    ps[:, :],
                    SCH_A,
                    SCH_B,
                    mybir.AluOpType.mult,
                    mybir.AluOpType.add,
                )
                # dummy scale pass over the bitcast fp32 terms to harvest
                # the row sum via the DVE accumulator
                k = n_sums[b]
                n_sums[b] += 1
                nc.vector.tensor_scalar(
                    exps_t[b][:, c0:c1],
                    sch_i32[:, 0:w].bitcast(f32),
                    1.0,
                    0.0,
                    mybir.AluOpType.mult,
                    mybir.AluOpType.add,
                    accum_out=sums_t[b][:, k : k + 1],
                )

            def last_prepare_bias():
                b = b_last
                nc.vector.reduce_sum(
                    rsum_t[b][:], sums_t[b][:, 0 : n_sums[b]],
                    axis=mybir.AxisListType.X,
                )
                nc.scalar.activation(
                    lnS[:], rsum_t[b][:], mybir.ActivationFunctionType.Ln
                )
                # -ln(S~) + ln(corr)
                nc.vector.tensor_scalar(
                    neg_lnS[:],
                    lnS[:],
                    -1.0,
                    SCH_LNCORR,
                    mybir.AluOpType.mult,
                    mybir.AluOpType.add,
                )

            def last_pass2_chunk(q):
                """PE chunk again -> ACT exp with bias=-ln(S): output is
                normalized as it leaves ACT; DMA the chunk immediately."""
                b = b_last
                c0, c1 = coff[q], coff[q + 1]
                ps = mm_chunk(b, c0, c1)
                nc.scalar.activation(
                    exps_t[b][:, c0:c1],
                    ps[:, :],
                    mybir.ActivationFunctionType.Exp,
                    scale=scale,
                    bias=neg_lnS[:],
                )
                nc.sync.dma_start(
                    out[b * 128 : (b + 1) * 128, c0:c1], exps_t[b][:, c0:c1]
                )

            # --- schedule ----------------------------------------------
            # Phase A: cover the input DMA stream.
            for i in range(len(q0_splits) - 1):
                for b in range(N_SPLIT):
                    do_cols(b, q0_splits[i], q0_splits[i + 1])
            for b in range(N_SPLIT, min(N_SPLIT + 1, n_blocks)):
                do_cols(b, 0, chunks[0])

            # Phase B: finish each block row, stream its output.  The last
            # block's pass-1 chunks are spread one at a time into the
            # middle of blocks 3-4 (away from the normalize bursts, so
            # their DVE work never holds a PSUM ring slot that the next
            # ACT chunk's matmuls are waiting for), and the bias prep runs
            # during block 5, long before pass 2 needs it.
            n_main = n_blocks - 1 if LN_LAST else n_blocks
            inserts = {}
            if LN_LAST:
                inserts = {
                    (3, 1): lambda: last_pass1_chunk(0),
                    (3, 2): lambda: last_pass1_chunk(1),
                    (4, 1): lambda: last_pass1_chunk(2),
                    (4, 2): lambda: last_pass1_chunk(3),
                    (5, 1): last_prepare_bias,
                }
            for b in range(n_main):
                q_start = 1 if b <= N_SPLIT else 0
                for q in range(q_start, n_q):
                    do_cols(b, coff[q], coff[q + 1])
                    ins = inserts.pop((b, q), None)
                    if ins is not None:
                        ins()
                if b >= n_main - 2:
                    n_dma = 4 if b == n_main - 1 else 2
                else:
                    n_dma = 1
                normalize_and_store(b, n_dma=n_dma)
            if LN_LAST:
                for q in range(n_q):
                    last_pass2_chunk(q)

    nc.compile()
    return nc


_nc_cache: dict = {}


def _get_nc():
    key = (MM_DT, N_WARM, N_SPLIT, LN_LAST)
    if key not in _nc_cache:
        _nc_cache[key] = build_nc()
    return _nc_cache[key]


def kernel(fmap1: np.ndarray, fmap2: np.ndarray) -> np.ndarray:
    f1 = np.asarray(fmap1, dtype=np.float32)
    f2 = np.asarray(fmap2, dtype=np.float32)
    np_mm = mybir.dt.np(getattr(mybir.dt, MM_DT))
    # [rows, D] -> [128, D/128, rows]: f1t[dp, dcc, n] = f1[n, dcc*128 + dp]
    f1t = np.ascontiguousarray(
        f1.T.reshape(DC, 128, N).transpose(1, 0, 2).astype(np_mm)
    )
    f2t = np.ascontiguousarray(
        f2.T.reshape(DC, 128, M).transpose(1, 0, 2).astype(np_mm)
    )

    nc = _get_nc()
    in_maps = [
        {"f1t": np.ascontiguousarray(f1t[:, :, i * NB : (i + 1) * NB]), "f2t": f2t}
        for i in range(N_CORES)
    ]
    trace = bool(os.environ.get("BASS_TRACE"))
    res = run_bass_kernel_spmd(nc, in_maps, list(range(N_CORES)), trace=trace)
    last_run_info.clear()
    last_run_info.update(
        exec_time_ns=res.exec_time_ns,
        mean_exec_time_ns=res.mean_exec_time_ns,
        profile_json=res.profile_json,
        trace_path=(res.instructions_and_trace or (None, None))[1],
    )
    return np.concatenate(
        [res.results[i]["out"] for i in range(N_CORES)], axis=0
    ).astype(np.float32)
